# revision 9
# baseline (speedup 1.0000x reference)
"""Trainium2 Bass kernel for Luong attention (nn_LuongAttention).

Reference computation (B=64, T=4096, U=256, D=256):
    state = decoder_s @ Wa_w + Wa_b                    # [B, U]
    score[b,t] = state[b] . encoder_h[b,t]             # [B, T]
    alpha = masked_softmax(score, mask)                # [B, T], zeros on padding
    ct_sum[b,u] = sum_t alpha[b,t] * encoder_h[b,t,u]  # [B, U]
    returns (ct_sum, alpha[..., None])

Sharding: pure data parallel, batch dim split 8 ways (8 batches/core).
Per-core design (memory-bound; encoder_h shard = 32MB read exactly once):
  - E tile layout [128, 32, 256], token t = p*32 + n (p=partition, n=chunk).
    Per-partition DMA runs are 32KB contiguous -> full HBM bandwidth.
  - score: fused multiply+row-reduce via scalar_tensor_tensor, chunks split
    between DVE and Pool engines.
  - softmax: masked via s2=(score+BIG)*mask, row reduce on DVE, cross-partition
    reduce on Pool (partition_all_reduce), exp+rowsum fused on ACT.
  - context: PE matmul accumulation over 32 chunks (lhsT = alpha column).
"""
import os
import sys

sys.path.insert(0, "/opt/trn_rl_repo")

from contextlib import ExitStack

import numpy as np

import concourse.bacc as bacc
import concourse.bass as bass
import concourse.bass_isa as bass_isa
import concourse.mybir as mybir
from concourse.bass_utils import run_bass_kernel_spmd
from concourse.tile import TileContext

F32 = mybir.dt.float32
I32 = mybir.dt.int32

N_CORES = 8
B, T, U, D = 64, 4096, 256, 256
BPC = B // N_CORES          # batches per core
NCH = T // 128              # 32 chunks of 128 tokens (t = p*32 + n)
BIG = 16384.0               # mask offset; exp(-BIG) underflows to exactly 0

# Score-chunk split between DVE and Pool engines.
N_DVE = int(os.environ.get("LUONG_NDVE", "32"))
# Context matmul dtype: fp32 (exact, 4 cyc/row) or float32r (1 cyc/row @ N>=256)
CT_F32R = os.environ.get("LUONG_CT_F32R", "0") == "1"


def _build():
    nc = bacc.Bacc(None, target_bir_lowering=False, debug=False)

    ENC = nc.dram_tensor("encoder_h", [BPC, T, U], F32, kind="ExternalInput")
    DEC = nc.dram_tensor("decoder_s", [BPC, D], F32, kind="ExternalInput")
    MASK = nc.dram_tensor("mask", [BPC, T], I32, kind="ExternalInput")
    WAW = nc.dram_tensor("Wa_w", [D, U], F32, kind="ExternalInput")
    WAB = nc.dram_tensor("Wa_b", [U], F32, kind="ExternalInput")
    CT = nc.dram_tensor("ct", [BPC, U], F32, kind="ExternalOutput")
    ALPHA = nc.dram_tensor("alpha", [BPC, T], F32, kind="ExternalOutput")

    mult = mybir.AluOpType.mult
    add = mybir.AluOpType.add
    is_equal = mybir.AluOpType.is_equal

    with TileContext(nc) as tc, ExitStack() as ctx:
        singles = ctx.enter_context(tc.tile_pool(name="singles", bufs=1))
        epool = ctx.enter_context(tc.tile_pool(name="epool", bufs=2))
        work = ctx.enter_context(tc.tile_pool(name="work", bufs=3))
        psum = ctx.enter_context(tc.tile_pool(name="psum", bufs=2, space="PSUM"))
        ctps = ctx.enter_context(tc.tile_pool(name="ctps", bufs=2, space="PSUM"))

        # ---- constants / small inputs ----
        w_sb = singles.tile([128, 2, U], F32)        # Wa_w, d = c*128 + p
        nc.sync.dma_start(w_sb, WAW.rearrange("(c p) u -> p c u", p=128))
        dec_sb = singles.tile([BPC, D], F32)
        nc.sync.dma_start(dec_sb, DEC[:])
        wab_sb = singles.tile([1, U], F32)
        nc.sync.dma_start(wab_sb, WAB.rearrange("(o u) -> o u", o=1))
        mask_all = singles.tile([128, BPC, NCH], I32)  # t = p*32 + n
        nc.sync.dma_start(mask_all, MASK.rearrange("b (p n) -> p b n", p=128))

        ones8 = singles.tile([1, BPC], F32)
        nc.vector.memset(ones8, 1.0)
        ident8 = singles.tile([BPC, BPC], F32)       # identity for dec transpose
        nc.vector.memset(ident8, 1.0)
        nc.gpsimd.affine_select(
            ident8, ident8, pattern=[[-1, BPC]], base=0, channel_multiplier=1,
            compare_op=is_equal, fill=0.0,
        )
        # sel[p, b, j] = 1 if p == b: one-hot lhsT used to broadcast state row b
        # to all 128 output partitions via the PE.
        sel = singles.tile([BPC, BPC, 128], F32)
        nc.vector.memset(sel, 1.0)
        nc.gpsimd.affine_select(
            sel, sel, pattern=[[-1, BPC], [0, 128]], base=0,
            channel_multiplier=1, compare_op=is_equal, fill=0.0,
        )

        # ---- state = decoder_s @ Wa_w + Wa_b : psum [8, U] ----
        state_ps = psum.tile([BPC, U], F32)
        # bias: broadcast Wa_b to all 8 rows (k=1 matmul with ones)
        nc.tensor.matmul(state_ps, ones8, wab_sb, start=True, stop=False)
        for c in range(2):
            dt_ps = psum.tile([128, BPC], F32)
            nc.tensor.transpose(dt_ps, dec_sb[:, c * 128:(c + 1) * 128], ident8)
            decT = work.tile([128, BPC], F32, tag="decT")
            nc.vector.tensor_copy(decT, dt_ps)
            nc.tensor.matmul(state_ps, decT, w_sb[:, c, :],
                             start=False, stop=(c == 1))
        state_sb = singles.tile([BPC, U], F32)
        nc.vector.tensor_copy(state_sb, state_ps)

        # broadcast each batch's state to all 128 partitions:
        # psum[j, u] = sum_p sel[p, b, j] * state_sb[p, u] = state_sb[b, u]
        state_bc = singles.tile([128, BPC, U], F32)
        for b in range(BPC):
            bc_ps = psum.tile([128, U], F32, tag="bc_ps")
            nc.tensor.matmul(bc_ps, sel[:, b, :], state_sb[:, :],
                             start=True, stop=True)
            nc.vector.tensor_copy(state_bc[:, b, :], bc_ps)

        ct_sb = singles.tile([1, BPC, U], F32)

        for b in range(BPC):
            et = epool.tile([128, NCH, U], F32)
            nc.sync.dma_start(et, ENC[b].rearrange("(p n) u -> p n u", p=128))

            score = work.tile([128, NCH], F32)
            junk_v = work.tile([128, U], F32, tag="junk_v")
            junk_p = work.tile([128, U], F32, tag="junk_p")
            for n in range(NCH):
                if n < N_DVE:
                    eng, junk = nc.vector, junk_v
                else:
                    eng, junk = nc.gpsimd, junk_p
                eng.scalar_tensor_tensor(
                    out=junk, in0=et[:, n, :], scalar=1.0,
                    in1=state_bc[:, b, :], op0=mult, op1=mult,
                    accum_out=score[:, n:n + 1])

            # Exact masked softmax. Shift by gmax' = max(max valid score, 0)
            # (softmax is shift invariant); mask applied AFTER exp so valid
            # scores are never quantized and padding is exactly 0.
            maskf = work.tile([128, NCH], F32)
            nc.vector.tensor_copy(maskf, mask_all[:, b, :])
            s2 = work.tile([128, NCH], F32)
            nc.vector.tensor_mul(s2, score, maskf)
            rowmax = work.tile([128, 1], F32)
            nc.vector.tensor_reduce(rowmax, s2, axis=mybir.AxisListType.X,
                                    op=mybir.AluOpType.max)
            gmax = work.tile([128, 1], F32)
            nc.gpsimd.partition_all_reduce(gmax, rowmax, channels=128,
                                           reduce_op=bass_isa.ReduceOp.max)
            negg = work.tile([128, 1], F32)
            nc.vector.tensor_scalar_mul(negg, gmax, -1.0)
            # Clamp padding scores to gmax so exp args stay <= 0 (HW exp
            # table misbehaves on large positive args; valid scores <= gmax).
            s3 = work.tile([128, NCH], F32)
            nc.vector.tensor_scalar_min(s3, score, gmax[:, :])
            prob_raw = work.tile([128, NCH], F32)
            nc.scalar.activation(prob_raw, s3,
                                 mybir.ActivationFunctionType.Exp,
                                 bias=negg[:, :], scale=1.0)
            prob = work.tile([128, NCH], F32)
            rowsum = work.tile([128, 1], F32)
            nc.vector.tensor_mul(prob, prob_raw, maskf)
            nc.vector.tensor_reduce(rowsum, prob, axis=mybir.AxisListType.X,
                                    op=mybir.AluOpType.add)
            gsum = work.tile([128, 1], F32)
            nc.gpsimd.partition_all_reduce(gsum, rowsum, channels=128,
                                           reduce_op=bass_isa.ReduceOp.add)
            rinv = work.tile([128, 1], F32)
            nc.vector.reciprocal(rinv, gsum)
            alpha_t = work.tile([128, NCH], F32)
            nc.vector.tensor_scalar_mul(alpha_t, prob, rinv[:, :])
            nc.sync.dma_start(ALPHA[b].rearrange("(p n) -> p n", p=128), alpha_t)

            # context: ct[u] = sum_t alpha[t] * E[t, u]
            ct_p = ctps.tile([1, U], F32)
            for n in range(NCH):
                lhsT = alpha_t[:, n:n + 1]
                rhs = et[:, n, :]
                if CT_F32R:
                    lhsT = lhsT.bitcast(mybir.dt.float32r)
                    rhs = rhs.bitcast(mybir.dt.float32r)
                nc.tensor.matmul(ct_p, lhsT, rhs,
                                 start=(n == 0), stop=(n == NCH - 1))
            nc.vector.tensor_copy(ct_sb[:, b, :], ct_p)

        nc.sync.dma_start(
            CT.rearrange("b u -> (b u)").rearrange("(o x) -> o x", o=1), ct_sb)

    nc.compile()
    return nc


_NC_CACHE = None


def _get_nc():
    global _NC_CACHE
    if _NC_CACHE is None:
        _NC_CACHE = _build()
    return _NC_CACHE


def kernel(**inputs):
    enc = np.ascontiguousarray(np.asarray(inputs["encoder_h"], dtype=np.float32))
    dec = np.ascontiguousarray(np.asarray(inputs["decoder_s"], dtype=np.float32))
    mask = np.ascontiguousarray(np.asarray(inputs["mask"], dtype=np.int32))
    waw = np.ascontiguousarray(np.asarray(inputs["Wa_w"], dtype=np.float32))
    wab = np.ascontiguousarray(np.asarray(inputs["Wa_b"], dtype=np.float32))

    nc = _get_nc()
    in_maps = []
    for c in range(N_CORES):
        sl = slice(c * BPC, (c + 1) * BPC)
        in_maps.append({
            "encoder_h": enc[sl], "decoder_s": dec[sl], "mask": mask[sl],
            "Wa_w": waw, "Wa_b": wab,
        })
    res = run_bass_kernel_spmd(nc, in_maps, core_ids=list(range(N_CORES)))
    ct = np.concatenate([r["ct"] for r in res.results], axis=0)
    alpha = np.concatenate([r["alpha"] for r in res.results], axis=0)
    return ct, alpha[..., None]


# revision 12
# speedup vs baseline: 1.3862x; 1.3862x over previous
"""Trainium2 Bass kernel for Luong attention (nn_LuongAttention).

Reference computation (B=64, T=4096, U=256, D=256):
    state = decoder_s @ Wa_w + Wa_b                    # [B, U]
    score[b,t] = state[b] . encoder_h[b,t]             # [B, T]
    alpha = masked_softmax(score, mask)                # [B, T], zeros on padding
    ct_sum[b,u] = sum_t alpha[b,t] * encoder_h[b,t,u]  # [B, U]
    returns (ct_sum, alpha[..., None])

Sharding: pure data parallel, batch dim split 8 ways (8 batches/core).
Per-core design (memory-bound; encoder_h shard = 32MB read exactly once):
  - E tile layout [128, 32, 256], token t = p*32 + n (p=partition, n=chunk).
    Per-partition DMA runs are 32KB contiguous -> full HBM bandwidth.
  - score: fused multiply+row-reduce via scalar_tensor_tensor, chunks split
    between DVE and Pool engines.
  - softmax: masked via s2=(score+BIG)*mask, row reduce on DVE, cross-partition
    reduce on Pool (partition_all_reduce), exp+rowsum fused on ACT.
  - context: PE matmul accumulation over 32 chunks (lhsT = alpha column).
"""
import os
import sys

sys.path.insert(0, "/opt/trn_rl_repo")

from contextlib import ExitStack

import numpy as np

import concourse.bacc as bacc
import concourse.bass as bass
import concourse.bass_isa as bass_isa
import concourse.mybir as mybir
from concourse.bass_utils import run_bass_kernel_spmd
from concourse.tile import TileContext

F32 = mybir.dt.float32
I32 = mybir.dt.int32

N_CORES = 8
B, T, U, D = 64, 4096, 256, 256
BPC = B // N_CORES          # batches per core
NCH = T // 128              # 32 chunks of 128 tokens (t = p*32 + n)
BIG = 16384.0               # mask offset; exp(-BIG) underflows to exactly 0

# Score-chunk split between DVE and Pool engines.
N_DVE = int(os.environ.get("LUONG_NDVE", "32"))
# Context matmul dtype: fp32 (exact, 4 cyc/row) or float32r (1 cyc/row @ N>=256)
CT_F32R = os.environ.get("LUONG_CT_F32R", "0") == "1"


def _build():
    nc = bacc.Bacc(None, target_bir_lowering=False, debug=False)

    ENC_DT = mybir.dt.float32r if CT_F32R else F32
    ENC = nc.dram_tensor("encoder_h", [BPC, T, U], ENC_DT, kind="ExternalInput")
    DEC = nc.dram_tensor("decoder_s", [BPC, D], F32, kind="ExternalInput")
    MASK = nc.dram_tensor("mask", [BPC, T], I32, kind="ExternalInput")
    WAW = nc.dram_tensor("Wa_w", [D, U], F32, kind="ExternalInput")
    WAB = nc.dram_tensor("Wa_b", [U], F32, kind="ExternalInput")
    CT = nc.dram_tensor("ct", [BPC, U], F32, kind="ExternalOutput")
    ALPHA = nc.dram_tensor("alpha", [BPC, T], F32, kind="ExternalOutput")

    mult = mybir.AluOpType.mult
    add = mybir.AluOpType.add
    is_equal = mybir.AluOpType.is_equal

    with TileContext(nc) as tc, ExitStack() as ctx:
        singles = ctx.enter_context(tc.tile_pool(name="singles", bufs=1))
        epool = ctx.enter_context(tc.tile_pool(name="epool", bufs=2))
        work = ctx.enter_context(tc.tile_pool(name="work", bufs=3))
        psum = ctx.enter_context(tc.tile_pool(name="psum", bufs=2, space="PSUM"))
        ctps = ctx.enter_context(tc.tile_pool(name="ctps", bufs=2, space="PSUM"))

        # ---- constants / small inputs ----
        w_sb = singles.tile([128, 2, U], F32)        # Wa_w, d = c*128 + p
        nc.sync.dma_start(w_sb, WAW.rearrange("(c p) u -> p c u", p=128))
        dec_sb = singles.tile([BPC, D], F32)
        nc.sync.dma_start(dec_sb, DEC[:])
        wab_sb = singles.tile([1, U], F32)
        nc.sync.dma_start(wab_sb, WAB.rearrange("(o u) -> o u", o=1))
        mask_all = singles.tile([128, BPC, NCH], I32)  # t = p*32 + n
        nc.sync.dma_start(mask_all, MASK.rearrange("b (p n) -> p b n", p=128))

        ones8 = singles.tile([1, BPC], F32)
        nc.vector.memset(ones8, 1.0)
        ident8 = singles.tile([BPC, BPC], F32)       # identity for dec transpose
        nc.vector.memset(ident8, 1.0)
        nc.gpsimd.affine_select(
            ident8, ident8, pattern=[[-1, BPC]], base=0, channel_multiplier=1,
            compare_op=is_equal, fill=0.0,
        )
        # sel[p, b, j] = 1 if p == b: one-hot lhsT used to broadcast state row b
        # to all 128 output partitions via the PE.
        sel = singles.tile([BPC, BPC, 128], F32)
        nc.vector.memset(sel, 1.0)
        nc.gpsimd.affine_select(
            sel, sel, pattern=[[-1, BPC], [0, 128]], base=0,
            channel_multiplier=1, compare_op=is_equal, fill=0.0,
        )

        # ---- state = decoder_s @ Wa_w + Wa_b : psum [8, U] ----
        state_ps = psum.tile([BPC, U], F32)
        # bias: broadcast Wa_b to all 8 rows (k=1 matmul with ones)
        nc.tensor.matmul(state_ps, ones8, wab_sb, start=True, stop=False)
        for c in range(2):
            dt_ps = psum.tile([128, BPC], F32)
            nc.tensor.transpose(dt_ps, dec_sb[:, c * 128:(c + 1) * 128], ident8)
            decT = work.tile([128, BPC], F32, tag="decT")
            nc.vector.tensor_copy(decT, dt_ps)
            nc.tensor.matmul(state_ps, decT, w_sb[:, c, :],
                             start=False, stop=(c == 1))
        state_sb = singles.tile([BPC, U], F32)
        nc.vector.tensor_copy(state_sb, state_ps)

        # broadcast each batch's state to all 128 partitions:
        # psum[j, u] = sum_p sel[p, b, j] * state_sb[p, u] = state_sb[b, u]
        state_bc = singles.tile([128, BPC, U], F32)
        for b in range(BPC):
            bc_ps = psum.tile([128, U], F32, tag="bc_ps")
            nc.tensor.matmul(bc_ps, sel[:, b, :], state_sb[:, :],
                             start=True, stop=True)
            nc.vector.tensor_copy(state_bc[:, b, :], bc_ps)

        ct_sb = singles.tile([1, BPC, U], F32)

        for b in range(BPC):
            et = epool.tile([128, NCH, U], ENC_DT)
            nc.sync.dma_start(et, ENC[b].rearrange("(p n) u -> p n u", p=128))

            score = work.tile([128, NCH], F32)
            junk_v = work.tile([128, U], F32, tag="junk_v")
            junk_p = work.tile([128, U], F32, tag="junk_p")
            for n in range(NCH):
                if n < N_DVE:
                    eng, junk = nc.vector, junk_v
                else:
                    eng, junk = nc.gpsimd, junk_p
                e_chunk = et[:, n, :]
                if CT_F32R:
                    e_chunk = e_chunk.bitcast(F32)
                eng.scalar_tensor_tensor(
                    out=junk, in0=e_chunk, scalar=1.0,
                    in1=state_bc[:, b, :], op0=mult, op1=mult,
                    accum_out=score[:, n:n + 1])

            # Exact masked softmax. Shift by gmax' = max(max valid score, 0)
            # (softmax is shift invariant); mask applied AFTER exp so valid
            # scores are never quantized and padding is exactly 0.
            maskf = work.tile([128, NCH], F32)
            nc.vector.tensor_copy(maskf, mask_all[:, b, :])
            s2 = work.tile([128, NCH], F32)
            nc.vector.tensor_mul(s2, score, maskf)
            rowmax = work.tile([128, 1], F32)
            nc.vector.tensor_reduce(rowmax, s2, axis=mybir.AxisListType.X,
                                    op=mybir.AluOpType.max)
            gmax = work.tile([128, 1], F32)
            nc.gpsimd.partition_all_reduce(gmax, rowmax, channels=128,
                                           reduce_op=bass_isa.ReduceOp.max)
            negg = work.tile([128, 1], F32)
            nc.vector.tensor_scalar_mul(negg, gmax, -1.0)
            # Clamp padding scores to gmax so exp args stay <= 0 (HW exp
            # table misbehaves on large positive args; valid scores <= gmax).
            s3 = work.tile([128, NCH], F32)
            nc.vector.tensor_scalar_min(s3, score, gmax[:, :])
            prob_raw = work.tile([128, NCH], F32)
            nc.scalar.activation(prob_raw, s3,
                                 mybir.ActivationFunctionType.Exp,
                                 bias=negg[:, :], scale=1.0)
            prob = work.tile([128, NCH], F32)
            rowsum = work.tile([128, 1], F32)
            nc.vector.tensor_mul(prob, prob_raw, maskf)
            nc.vector.tensor_reduce(rowsum, prob, axis=mybir.AxisListType.X,
                                    op=mybir.AluOpType.add)
            gsum = work.tile([128, 1], F32)
            nc.gpsimd.partition_all_reduce(gsum, rowsum, channels=128,
                                           reduce_op=bass_isa.ReduceOp.add)
            rinv = work.tile([128, 1], F32)
            nc.vector.reciprocal(rinv, gsum)
            alpha_t = work.tile([128, NCH], F32)
            nc.vector.tensor_scalar_mul(alpha_t, prob, rinv[:, :])
            nc.sync.dma_start(ALPHA[b].rearrange("(p n) -> p n", p=128), alpha_t)

            # context: ct[u] = sum_t alpha[t] * E[t, u]
            if CT_F32R:
                alpha_r = work.tile([128, NCH], mybir.dt.float32r)
                nc.vector.tensor_copy(alpha_r, alpha_t)
                alpha_lhs = alpha_r
            else:
                alpha_lhs = alpha_t
            ct_p = ctps.tile([1, U], F32)
            for n in range(NCH):
                nc.tensor.matmul(ct_p, alpha_lhs[:, n:n + 1], et[:, n, :],
                                 start=(n == 0), stop=(n == NCH - 1))
            nc.vector.tensor_copy(ct_sb[:, b, :], ct_p)

        nc.sync.dma_start(
            CT.rearrange("b u -> (b u)").rearrange("(o x) -> o x", o=1), ct_sb)

    nc.compile()
    return nc


_NC_CACHE = None


def _get_nc():
    global _NC_CACHE
    if _NC_CACHE is None:
        _NC_CACHE = _build()
    return _NC_CACHE


def kernel(**inputs):
    enc = np.ascontiguousarray(np.asarray(inputs["encoder_h"], dtype=np.float32))
    dec = np.ascontiguousarray(np.asarray(inputs["decoder_s"], dtype=np.float32))
    mask = np.ascontiguousarray(np.asarray(inputs["mask"], dtype=np.int32))
    waw = np.ascontiguousarray(np.asarray(inputs["Wa_w"], dtype=np.float32))
    wab = np.ascontiguousarray(np.asarray(inputs["Wa_b"], dtype=np.float32))

    nc = _get_nc()
    in_maps = []
    for c in range(N_CORES):
        sl = slice(c * BPC, (c + 1) * BPC)
        in_maps.append({
            "encoder_h": enc[sl], "decoder_s": dec[sl], "mask": mask[sl],
            "Wa_w": waw, "Wa_b": wab,
        })
    res = run_bass_kernel_spmd(nc, in_maps, core_ids=list(range(N_CORES)))
    ct = np.concatenate([r["ct"] for r in res.results], axis=0)
    alpha = np.concatenate([r["alpha"] for r in res.results], axis=0)
    return ct, alpha[..., None]


# revision 20
# speedup vs baseline: 1.7740x; 1.2797x over previous
"""Trainium2 Bass kernel for Luong attention (nn_LuongAttention).

Reference computation (B=64, T=4096, U=256, D=256):
    state = decoder_s @ Wa_w + Wa_b                    # [B, U]
    score[b,t] = state[b] . encoder_h[b,t]             # [B, T]
    alpha = masked_softmax(score, mask)                # [B, T], zeros on padding
    ct_sum[b,u] = sum_t alpha[b,t] * encoder_h[b,t,u]  # [B, U]
    returns (ct_sum, alpha[..., None])

Sharding: pure data parallel, batch dim split 8 ways (8 batches/core).
Per-core design (memory-bound; encoder_h shard = 32MB read exactly once):
  - E tile layout [128, 32, 256], token t = p*32 + n (p=partition, n=chunk).
    Per-partition DMA runs are 32KB contiguous -> full HBM bandwidth.
  - score: fused multiply+row-reduce via scalar_tensor_tensor, chunks split
    between DVE and Pool engines.
  - softmax: masked via s2=(score+BIG)*mask, row reduce on DVE, cross-partition
    reduce on Pool (partition_all_reduce), exp+rowsum fused on ACT.
  - context: PE matmul accumulation over 32 chunks (lhsT = alpha column).
"""
import os
import sys

sys.path.insert(0, "/opt/trn_rl_repo")

from contextlib import ExitStack

import numpy as np

import concourse.bacc as bacc
import concourse.bass as bass
import concourse.bass_isa as bass_isa
import concourse.mybir as mybir
from concourse.bass_utils import run_bass_kernel_spmd
from concourse.tile import TileContext

F32 = mybir.dt.float32
I32 = mybir.dt.int32

N_CORES = 8
B, T, U, D = 64, 4096, 256, 256
BPC = B // N_CORES          # batches per core
NCH = T // 128              # 32 chunks of 128 tokens (t = p*32 + n)
BIG = 16384.0               # mask offset; exp(-BIG) underflows to exactly 0

# Score-chunk split between DVE and Pool engines.
N_DVE = int(os.environ.get("LUONG_NDVE", "32"))
# Context matmul dtype: fp32 (exact, 4 cyc/row) or float32r (1 cyc/row @ N>=256)
CT_F32R = os.environ.get("LUONG_CT_F32R", "0") == "1"
# Ablation: dma | score | softmax | full
ABLATE = os.environ.get("LUONG_ABLATE", "full")


def _build(reps=1):
    nc = bacc.Bacc(None, target_bir_lowering=False, debug=False)

    ENC_DT = mybir.dt.float32r if CT_F32R else F32
    ENC = nc.dram_tensor("encoder_h", [BPC, T, U], ENC_DT, kind="ExternalInput")
    DEC = nc.dram_tensor("decoder_s", [BPC, D], F32, kind="ExternalInput")
    MASK = nc.dram_tensor("mask", [BPC, T], I32, kind="ExternalInput")
    WAW = nc.dram_tensor("Wa_w", [D, U], F32, kind="ExternalInput")
    WAB = nc.dram_tensor("Wa_b", [U], F32, kind="ExternalInput")
    CT = nc.dram_tensor("ct", [BPC, U], F32, kind="ExternalOutput")
    ALPHA = nc.dram_tensor("alpha", [BPC, T], F32, kind="ExternalOutput")

    mult = mybir.AluOpType.mult
    add = mybir.AluOpType.add
    is_equal = mybir.AluOpType.is_equal

    with TileContext(nc) as tc, ExitStack() as ctx:
        singles = ctx.enter_context(tc.tile_pool(name="singles", bufs=1))
        epool = ctx.enter_context(tc.tile_pool(name="epool", bufs=2))
        work = ctx.enter_context(tc.tile_pool(name="work", bufs=3))
        psum = ctx.enter_context(tc.tile_pool(name="psum", bufs=2, space="PSUM"))
        ctps = ctx.enter_context(tc.tile_pool(name="ctps", bufs=2, space="PSUM"))

        # ---- constants / small inputs ----
        w_sb = singles.tile([128, 2, U], F32)        # Wa_w, d = c*128 + p
        nc.sync.dma_start(w_sb, WAW.rearrange("(c p) u -> p c u", p=128))
        dec_sb = singles.tile([BPC, D], F32)
        nc.sync.dma_start(dec_sb, DEC[:])
        wab_sb = singles.tile([1, U], F32)
        nc.sync.dma_start(wab_sb, WAB.rearrange("(o u) -> o u", o=1))
        mask_all = singles.tile([128, BPC, NCH], I32)  # t = p*32 + n
        nc.sync.dma_start(mask_all, MASK.rearrange("b (p n) -> p b n", p=128))

        ones8 = singles.tile([1, BPC], F32)
        nc.vector.memset(ones8, 1.0)
        ident8 = singles.tile([BPC, BPC], F32)       # identity for dec transpose
        nc.vector.memset(ident8, 1.0)
        nc.gpsimd.affine_select(
            ident8, ident8, pattern=[[-1, BPC]], base=0, channel_multiplier=1,
            compare_op=is_equal, fill=0.0,
        )
        # sel[p, b, j] = 1 if p == b: one-hot lhsT used to broadcast state row b
        # to all 128 output partitions via the PE.
        sel = singles.tile([BPC, BPC, 128], F32)
        nc.vector.memset(sel, 1.0)
        nc.gpsimd.affine_select(
            sel, sel, pattern=[[-1, BPC], [0, 128]], base=0,
            channel_multiplier=1, compare_op=is_equal, fill=0.0,
        )

        # ---- state = decoder_s @ Wa_w + Wa_b : psum [8, U] ----
        state_ps = psum.tile([BPC, U], F32)
        # bias: broadcast Wa_b to all 8 rows (k=1 matmul with ones)
        nc.tensor.matmul(state_ps, ones8, wab_sb, start=True, stop=False)
        for c in range(2):
            dt_ps = psum.tile([128, BPC], F32)
            nc.tensor.transpose(dt_ps, dec_sb[:, c * 128:(c + 1) * 128], ident8)
            decT = work.tile([128, BPC], F32, tag="decT")
            nc.vector.tensor_copy(decT, dt_ps)
            nc.tensor.matmul(state_ps, decT, w_sb[:, c, :],
                             start=False, stop=(c == 1))
        state_sb = singles.tile([BPC, U], F32)
        nc.vector.tensor_copy(state_sb, state_ps)

        # broadcast each batch's state to all 128 partitions:
        # psum[j, u] = sum_p sel[p, b, j] * state_sb[p, u] = state_sb[b, u]
        state_bc = singles.tile([128, BPC, U], F32)
        for b in range(BPC):
            bc_ps = psum.tile([128, U], F32, tag="bc_ps")
            nc.tensor.matmul(bc_ps, sel[:, b, :], state_sb[:, :],
                             start=True, stop=True)
            nc.vector.tensor_copy(state_bc[:, b, :], bc_ps)

        ct_sb = singles.tile([1, BPC, U], F32)
        if ABLATE != "full":
            nc.vector.memset(ct_sb, 0.0)

        rep_cm = tc.For_i(0, reps, 1) if reps > 1 else None
        if rep_cm is not None:
            rep_cm.__enter__()
        NG = 4                  # E-load split: NG sub-DMAs per batch
        GCH = NCH // NG         # chunks per group
        enc_r = ENC.rearrange("b (p g n) u -> b p g n u", p=128, g=NG)

        for b in range(BPC):
            ets = []
            for g in range(NG):
                etg = epool.tile([128, GCH, U], ENC_DT, tag=f"et_g{g}")
                nc.sync.dma_start(etg, enc_r[b, :, g, :, :])
                ets.append(etg)

            def echunk(n):
                c = ets[n // GCH][:, n % GCH, :]
                return c.bitcast(F32) if CT_F32R else c

            if ABLATE == "dma":
                # consume one element so the load isn't dead
                tick = work.tile([128, 1], F32, tag="tick")
                nc.vector.tensor_copy(tick, echunk(0)[:, 0:1])
                nc.scalar.dma_start(
                    ALPHA[b].rearrange("(p n) -> p n", p=128)[:, 0:1], tick)
                continue

            score = work.tile([128, NCH], F32)
            junk_v = work.tile([128, U], F32, tag="junk_v")
            junk_p = work.tile([128, U], F32, tag="junk_p")
            for n in range(NCH):
                if n < N_DVE:
                    eng, junk = nc.vector, junk_v
                else:
                    eng, junk = nc.gpsimd, junk_p
                eng.scalar_tensor_tensor(
                    out=junk, in0=echunk(n), scalar=1.0,
                    in1=state_bc[:, b, :], op0=mult, op1=mult,
                    accum_out=score[:, n:n + 1])

            if ABLATE == "score":
                nc.scalar.dma_start(ALPHA[b].rearrange("(p n) -> p n", p=128), score)
                continue

            # Exact masked softmax. Shift by gmax' = max(max valid score, 0)
            # (softmax is shift invariant); mask applied AFTER exp so valid
            # scores are never quantized and padding is exactly 0.
            maskf = work.tile([128, NCH], F32)
            nc.vector.tensor_copy(maskf, mask_all[:, b, :])
            s2 = work.tile([128, NCH], F32)
            nc.vector.tensor_mul(s2, score, maskf)
            rowmax = work.tile([128, 1], F32)
            nc.vector.tensor_reduce(rowmax, s2, axis=mybir.AxisListType.X,
                                    op=mybir.AluOpType.max)
            gmax = work.tile([128, 1], F32)
            nc.gpsimd.partition_all_reduce(gmax, rowmax, channels=128,
                                           reduce_op=bass_isa.ReduceOp.max)
            negg = work.tile([128, 1], F32)
            nc.vector.tensor_scalar_mul(negg, gmax, -1.0)
            # Clamp padding scores to gmax so exp args stay <= 0 (HW exp
            # table misbehaves on large positive args; valid scores <= gmax).
            s3 = work.tile([128, NCH], F32)
            nc.vector.tensor_scalar_min(s3, score, gmax[:, :])
            prob_raw = work.tile([128, NCH], F32)
            nc.scalar.activation(prob_raw, s3,
                                 mybir.ActivationFunctionType.Exp,
                                 bias=negg[:, :], scale=1.0)
            prob = work.tile([128, NCH], F32)
            rowsum = work.tile([128, 1], F32)
            nc.vector.tensor_mul(prob, prob_raw, maskf)
            nc.vector.tensor_reduce(rowsum, prob, axis=mybir.AxisListType.X,
                                    op=mybir.AluOpType.add)
            gsum = work.tile([128, 1], F32)
            nc.gpsimd.partition_all_reduce(gsum, rowsum, channels=128,
                                           reduce_op=bass_isa.ReduceOp.add)
            rinv = work.tile([128, 1], F32)
            nc.vector.reciprocal(rinv, gsum)
            alpha_t = work.tile([128, NCH], F32)
            nc.vector.tensor_scalar_mul(alpha_t, prob, rinv[:, :])
            nc.scalar.dma_start(ALPHA[b].rearrange("(p n) -> p n", p=128), alpha_t)

            if ABLATE == "softmax":
                continue

            # context: ct[u] = sum_t alpha[t] * E[t, u]
            if CT_F32R:
                alpha_r = work.tile([128, NCH], mybir.dt.float32r)
                nc.vector.tensor_copy(alpha_r, alpha_t)
                alpha_lhs = alpha_r
            else:
                alpha_lhs = alpha_t
            ct_p = ctps.tile([1, U], F32)
            for n in range(NCH):
                nc.tensor.matmul(ct_p, alpha_lhs[:, n:n + 1],
                                 ets[n // GCH][:, n % GCH, :],
                                 start=(n == 0), stop=(n == NCH - 1))
            nc.vector.tensor_copy(ct_sb[:, b, :], ct_p)

        nc.scalar.dma_start(
            CT.rearrange("b u -> (b u)").rearrange("(o x) -> o x", o=1), ct_sb)
        if rep_cm is not None:
            rep_cm.__exit__(None, None, None)

    nc.compile()
    return nc


_NC_CACHE = None


def _get_nc():
    global _NC_CACHE
    if _NC_CACHE is None:
        _NC_CACHE = _build()
    return _NC_CACHE


def kernel(**inputs):
    enc = np.ascontiguousarray(np.asarray(inputs["encoder_h"], dtype=np.float32))
    dec = np.ascontiguousarray(np.asarray(inputs["decoder_s"], dtype=np.float32))
    mask = np.ascontiguousarray(np.asarray(inputs["mask"], dtype=np.int32))
    waw = np.ascontiguousarray(np.asarray(inputs["Wa_w"], dtype=np.float32))
    wab = np.ascontiguousarray(np.asarray(inputs["Wa_b"], dtype=np.float32))

    nc = _get_nc()
    in_maps = []
    for c in range(N_CORES):
        sl = slice(c * BPC, (c + 1) * BPC)
        in_maps.append({
            "encoder_h": enc[sl], "decoder_s": dec[sl], "mask": mask[sl],
            "Wa_w": waw, "Wa_b": wab,
        })
    res = run_bass_kernel_spmd(nc, in_maps, core_ids=list(range(N_CORES)))
    ct = np.concatenate([r["ct"] for r in res.results], axis=0)
    alpha = np.concatenate([r["alpha"] for r in res.results], axis=0)
    return ct, alpha[..., None]


# revision 23
# speedup vs baseline: 1.8549x; 1.0456x over previous
"""Trainium2 Bass kernel for Luong attention (nn_LuongAttention).

Reference computation (B=64, T=4096, U=256, D=256):
    state = decoder_s @ Wa_w + Wa_b                    # [B, U]
    score[b,t] = state[b] . encoder_h[b,t]             # [B, T]
    alpha = masked_softmax(score, mask)                # [B, T], zeros on padding
    ct_sum[b,u] = sum_t alpha[b,t] * encoder_h[b,t,u]  # [B, U]
    returns (ct_sum, alpha[..., None])

Sharding: pure data parallel, batch dim split 8 ways (8 batches/core).

Per-core design (memory regime; the 32MB encoder_h shard is read exactly once):
  - E tile layout [128, 32, 256], token t = p*32 + n (p=partition, n=chunk),
    loaded as 4 sub-DMAs per batch so scoring starts before the full batch
    lands. Per-partition runs are 8KB contiguous -> full HBM bandwidth.
  - score: fused multiply+row-reduce (scalar_tensor_tensor) on DVE; every
    third chunk is offloaded as Pool multiply + ACT copy-with-accumulate.
  - softmax: exact (mask applied after exp; shift by max(valid, 0); padding
    exp args clamped). Cross-partition reduces on Pool (partition_all_reduce).
  - context: PE fp32 matmul accumulation (lhsT = alpha column, rhs = E chunk).
  - Emission is software-pipelined: batch b's softmax/context ops are
    interleaved into batch b+1's score stream so cross-engine latency hides
    behind independent DVE work.
"""
import os
import sys

sys.path.insert(0, "/opt/trn_rl_repo")

from contextlib import ExitStack

import numpy as np

import concourse.bacc as bacc
import concourse.bass as bass
import concourse.bass_isa as bass_isa
import concourse.mybir as mybir
from concourse.bass_utils import run_bass_kernel_spmd
from concourse.tile import TileContext

F32 = mybir.dt.float32
I32 = mybir.dt.int32

N_CORES = 8
B, T, U, D = 64, 4096, 256, 256
BPC = B // N_CORES          # batches per core
NCH = T // 128              # 32 chunks of 128 tokens (t = p*32 + n)
NG = 4                      # E-load split: NG sub-DMAs per batch
GCH = NCH // NG

# Every POOL_EVERY-th score chunk goes Pool(mult)+ACT(reduce); 0 = all DVE.
POOL_EVERY = int(os.environ.get("LUONG_POOL_EVERY", "3"))
# Context matmul dtype: fp32 (exact, 4 cyc/row) or float32r (1 cyc/row)
CT_F32R = os.environ.get("LUONG_CT_F32R", "0") == "1"
# Ablation: dma | full
ABLATE = os.environ.get("LUONG_ABLATE", "full")
EPOOL_BUFS = int(os.environ.get("LUONG_EPOOL_BUFS", "3"))


def _build(reps=1):
    nc = bacc.Bacc(None, target_bir_lowering=False, debug=False)

    ENC_DT = mybir.dt.float32r if CT_F32R else F32
    ENC = nc.dram_tensor("encoder_h", [BPC, T, U], ENC_DT, kind="ExternalInput")
    DEC = nc.dram_tensor("decoder_s", [BPC, D], F32, kind="ExternalInput")
    MASK = nc.dram_tensor("mask", [BPC, T], I32, kind="ExternalInput")
    WAW = nc.dram_tensor("Wa_w", [D, U], F32, kind="ExternalInput")
    WAB = nc.dram_tensor("Wa_b", [U], F32, kind="ExternalInput")
    CT = nc.dram_tensor("ct", [BPC, U], F32, kind="ExternalOutput")
    ALPHA = nc.dram_tensor("alpha", [BPC, T], F32, kind="ExternalOutput")

    mult = mybir.AluOpType.mult
    is_equal = mybir.AluOpType.is_equal
    Copy = mybir.ActivationFunctionType.Copy
    Exp = mybir.ActivationFunctionType.Exp

    with TileContext(nc) as tc, ExitStack() as ctx:
        singles = ctx.enter_context(tc.tile_pool(name="singles", bufs=1))
        epool = ctx.enter_context(tc.tile_pool(name="epool", bufs=EPOOL_BUFS))
        work = ctx.enter_context(tc.tile_pool(name="work", bufs=3))
        jpool = ctx.enter_context(tc.tile_pool(name="jpool", bufs=4))
        psum = ctx.enter_context(tc.tile_pool(name="psum", bufs=2, space="PSUM"))
        ctps = ctx.enter_context(tc.tile_pool(name="ctps", bufs=2, space="PSUM"))

        # ---- constants / small inputs ----
        w_sb = singles.tile([128, 2, U], F32)        # Wa_w, d = c*128 + p
        nc.sync.dma_start(w_sb, WAW.rearrange("(c p) u -> p c u", p=128))
        dec_sb = singles.tile([BPC, D], F32)
        nc.sync.dma_start(dec_sb, DEC[:])
        wab_sb = singles.tile([1, U], F32)
        nc.sync.dma_start(wab_sb, WAB.rearrange("(o u) -> o u", o=1))
        mask_i = singles.tile([128, BPC, NCH], I32)  # t = p*32 + n
        nc.sync.dma_start(mask_i, MASK.rearrange("b (p n) -> p b n", p=128))
        mask_f = singles.tile([128, BPC, NCH], F32)
        nc.vector.tensor_copy(mask_f, mask_i)

        ones8 = singles.tile([1, BPC], F32)
        nc.vector.memset(ones8, 1.0)
        ident8 = singles.tile([BPC, BPC], F32)       # identity for dec transpose
        nc.vector.memset(ident8, 1.0)
        nc.gpsimd.affine_select(
            ident8, ident8, pattern=[[-1, BPC]], base=0, channel_multiplier=1,
            compare_op=is_equal, fill=0.0,
        )
        # sel[p, b, j] = 1 if p == b: one-hot lhsT used to broadcast state row b
        # to all 128 output partitions via the PE.
        sel = singles.tile([BPC, BPC, 128], F32)
        nc.vector.memset(sel, 1.0)
        nc.gpsimd.affine_select(
            sel, sel, pattern=[[-1, BPC], [0, 128]], base=0,
            channel_multiplier=1, compare_op=is_equal, fill=0.0,
        )

        # ---- state = decoder_s @ Wa_w + Wa_b : psum [8, U] ----
        state_ps = psum.tile([BPC, U], F32)
        nc.tensor.matmul(state_ps, ones8, wab_sb, start=True, stop=False)
        for c in range(2):
            dt_ps = psum.tile([128, BPC], F32)
            nc.tensor.transpose(dt_ps, dec_sb[:, c * 128:(c + 1) * 128], ident8)
            decT = work.tile([128, BPC], F32, tag="decT")
            nc.vector.tensor_copy(decT, dt_ps)
            nc.tensor.matmul(state_ps, decT, w_sb[:, c, :],
                             start=False, stop=(c == 1))
        state_sb = singles.tile([BPC, U], F32)
        nc.vector.tensor_copy(state_sb, state_ps)

        # broadcast each batch's state to all 128 partitions:
        # psum[j, u] = sum_p sel[p, b, j] * state_sb[p, u] = state_sb[b, u]
        state_bc = singles.tile([128, BPC, U], F32)
        for b in range(BPC):
            bc_ps = psum.tile([128, U], F32, tag="bc_ps")
            nc.tensor.matmul(bc_ps, sel[:, b, :], state_sb[:, :],
                             start=True, stop=True)
            nc.vector.tensor_copy(state_bc[:, b, :], bc_ps)

        ct_sb = singles.tile([1, BPC, U], F32)
        if ABLATE != "full":
            nc.vector.memset(ct_sb, 0.0)

        rep_cm = tc.For_i(0, reps, 1) if reps > 1 else None
        if rep_cm is not None:
            rep_cm.__enter__()

        enc_r = ENC.rearrange("b (p g n) u -> b p g n u", p=128, g=NG)
        alpha_r = ALPHA.rearrange("b (p n) -> b p n", p=128)

        state_b = {}     # per-batch tiles threaded between pipeline stages

        def emit_loads(b):
            ets = []
            for g in range(NG):
                etg = epool.tile([128, GCH, U], ENC_DT, tag=f"et_g{g}", name=f"et_g{g}")
                nc.sync.dma_start(etg, enc_r[b, :, g, :, :])
                ets.append(etg)
            state_b[b] = {"ets": ets}

        def echunk(b, n):
            c = state_b[b]["ets"][n // GCH][:, n % GCH, :]
            return c.bitcast(F32) if CT_F32R else c

        def emit_score_chunk(b, n):
            st = state_b[b]
            if n == 0:
                st["score"] = work.tile([128, NCH], F32, tag="score", name="score")
                st["junk_v"] = work.tile([128, U], F32, tag="junk_v", name="junk_v")
            if POOL_EVERY and (n % POOL_EVERY == POOL_EVERY - 1):
                junk_p = jpool.tile([128, U], F32, tag="junk_p", name="junk_p")
                junk_a = work.tile([128, U], F32, tag="junk_a", name="junk_a")
                nc.gpsimd.tensor_mul(junk_p, echunk(b, n), state_bc[:, b, :])
                nc.scalar.activation(junk_a, junk_p, Copy,
                                     accum_out=st["score"][:, n:n + 1])
            else:
                nc.vector.scalar_tensor_tensor(
                    out=st["junk_v"], in0=echunk(b, n), scalar=1.0,
                    in1=state_bc[:, b, :], op0=mult, op1=mult,
                    accum_out=st["score"][:, n:n + 1])

        def finish_ops(b):
            """Softmax + context for batch b as a list of (name, thunk)."""
            st = state_b[b]
            mk = lambda shape, tag: work.tile(shape, F32, tag=tag, name=tag)

            def f_s2():
                st["s2"] = mk([128, NCH], "s2")
                nc.vector.tensor_mul(st["s2"], st["score"], mask_f[:, b, :])

            def f_rowmax():
                st["rowmax"] = mk([128, 1], "rowmax")
                nc.vector.tensor_reduce(st["rowmax"], st["s2"],
                                        axis=mybir.AxisListType.X,
                                        op=mybir.AluOpType.max)

            def f_gmax():
                st["gmax"] = mk([128, 1], "gmax")
                nc.gpsimd.partition_all_reduce(st["gmax"], st["rowmax"],
                                               channels=128,
                                               reduce_op=bass_isa.ReduceOp.max)

            def f_negg():
                st["negg"] = mk([128, 1], "negg")
                nc.scalar.activation(st["negg"], st["gmax"], Copy, scale=-1.0)

            def f_s3():
                st["s3"] = mk([128, NCH], "s3")
                nc.vector.tensor_scalar_min(st["s3"], st["score"],
                                            st["gmax"][:, :])

            def f_exp():
                st["praw"] = mk([128, NCH], "praw")
                nc.scalar.activation(st["praw"], st["s3"], Exp,
                                     bias=st["negg"][:, :], scale=1.0)

            def f_prob():
                st["prob"] = mk([128, NCH], "prob")
                st["rowsum"] = mk([128, 1], "rowsum")
                nc.vector.scalar_tensor_tensor(
                    out=st["prob"], in0=st["praw"], scalar=1.0,
                    in1=mask_f[:, b, :], op0=mult, op1=mult,
                    accum_out=st["rowsum"])

            def f_gsum():
                st["gsum"] = mk([128, 1], "gsum")
                nc.gpsimd.partition_all_reduce(st["gsum"], st["rowsum"],
                                               channels=128,
                                               reduce_op=bass_isa.ReduceOp.add)

            def f_rinv():
                st["rinv"] = mk([128, 1], "rinv")
                nc.vector.reciprocal(st["rinv"], st["gsum"])

            def f_alpha():
                st["alpha"] = mk([128, NCH], "alpha")
                nc.vector.tensor_scalar_mul(st["alpha"], st["prob"],
                                            st["rinv"][:, :])
                if CT_F32R:
                    ar = work.tile([128, NCH], mybir.dt.float32r, tag="alphar", name="alphar")
                    nc.vector.tensor_copy(ar, st["alpha"])
                    st["alpha_mm"] = ar
                else:
                    st["alpha_mm"] = st["alpha"]

            def f_store():
                nc.scalar.dma_start(alpha_r[b], st["alpha"])

            def f_ct():
                ct_p = ctps.tile([1, U], F32, name="ct_p")
                for n in range(NCH):
                    nc.tensor.matmul(ct_p, st["alpha_mm"][:, n:n + 1],
                                     state_b[b]["ets"][n // GCH][:, n % GCH, :],
                                     start=(n == 0), stop=(n == NCH - 1))
                st["ct_p"] = ct_p

            def f_ctcopy():
                nc.scalar.activation(ct_sb[:, b, :], st["ct_p"], Copy)
                del state_b[b]

            return [f_s2, f_rowmax, f_gmax, f_negg, f_s3, f_exp, f_prob,
                    f_gsum, f_rinv, f_alpha, f_store, f_ct, f_ctcopy]

        # insertion marks: after score-chunk j of the NEXT batch, run the i-th
        # finish op of the PREVIOUS batch
        MARKS = [2, 4, 6, 8, 10, 12, 16, 18, 22, 24, 25, 26, 31]

        def emit_batch_scores(b, pending):
            """Emit score chunks for batch b, interleaving pending finish ops."""
            mi = 0
            for n in range(NCH):
                emit_score_chunk(b, n)
                while (pending is not None and mi < len(MARKS)
                       and n >= MARKS[mi]):
                    pending[mi]()
                    mi += 1
            if pending is not None:
                while mi < len(pending):
                    pending[mi]()
                    mi += 1

        if ABLATE == "dma":
            for b in range(BPC):
                emit_loads(b)
                tick = work.tile([128, 1], F32, tag="tick")
                nc.vector.tensor_copy(tick, echunk(b, 0)[:, 0:1])
                nc.scalar.dma_start(alpha_r[b][:, 0:1], tick)
                del state_b[b]
        else:
            emit_loads(0)
            pending = None
            for b in range(BPC):
                if b + 1 < BPC:
                    emit_loads(b + 1)
                emit_batch_scores(b, pending)
                pending = finish_ops(b)
            for f in pending:
                f()

        if rep_cm is not None:
            rep_cm.__exit__(None, None, None)

        nc.scalar.dma_start(
            CT.rearrange("b u -> (b u)").rearrange("(o x) -> o x", o=1), ct_sb)

    nc.compile()
    return nc


_NC_CACHE = None


def _get_nc():
    global _NC_CACHE
    if _NC_CACHE is None:
        _NC_CACHE = _build()
    return _NC_CACHE


def kernel(**inputs):
    enc = np.ascontiguousarray(np.asarray(inputs["encoder_h"], dtype=np.float32))
    dec = np.ascontiguousarray(np.asarray(inputs["decoder_s"], dtype=np.float32))
    mask = np.ascontiguousarray(np.asarray(inputs["mask"], dtype=np.int32))
    waw = np.ascontiguousarray(np.asarray(inputs["Wa_w"], dtype=np.float32))
    wab = np.ascontiguousarray(np.asarray(inputs["Wa_b"], dtype=np.float32))

    nc = _get_nc()
    in_maps = []
    for c in range(N_CORES):
        sl = slice(c * BPC, (c + 1) * BPC)
        in_maps.append({
            "encoder_h": enc[sl], "decoder_s": dec[sl], "mask": mask[sl],
            "Wa_w": waw, "Wa_b": wab,
        })
    res = run_bass_kernel_spmd(nc, in_maps, core_ids=list(range(N_CORES)))
    ct = np.concatenate([r["ct"] for r in res.results], axis=0)
    alpha = np.concatenate([r["alpha"] for r in res.results], axis=0)
    return ct, alpha[..., None]
